# revision 29
# baseline (speedup 1.0000x reference)
"""Trainium2 Bass kernel for nn_RankingLoss (pairwise hinge ranking loss).

reference semantics (N = 8192):
    d = targets[:,0]; e = targets[:,1]
    valid[i,j] = (d[i] < d[j]) & (e[i] == 1)
    hinge[i,j] = relu(1.0 - (p[i] - p[j]))
    loss = sum(valid*hinge) / max(sum(valid), 1)   (0 if no pairs)

Device algorithm (per core, j-axis sharded across 8 cores, both axes sorted
by duration on the host — an O(N log N) relabeling, like causal masking):

  After sorting, [d_i < d_j] is a rank triangle up to exact-tie noise, so
  for an i-chunk strictly below a j-tile's rank range the mask is just e_i,
  and for a chunk strictly above it is 0 (those ops and matmuls are simply
  skipped — ~75% of the pairwise work is provably zero).  Only the chunk
  containing the tile's own ranks (its "diagonal" chunk) computes the exact
  f32 duration compare.

  Layout: partition axis = j (128 per tile; core c's tile t covers sorted
  ranks [1024 t + 128 c, +128) so every core touches all rank levels and the
  load is balanced), free axis = i (4 macro-chunks of 2048).  The i-axis
  vectors are broadcast across partitions with a K=16 TensorE matmul over
  16 host-replicated rows (the sum scales values by exactly 16, which is
  folded into the j-side scalars; 16 rows make the input DMA fast), then one
  engine copy per chunk — keeping every hot-loop dependency a single-engine
  semaphore (walrus fits only one sync wait on LDWEIGHTS).

  We[j,i] = [16 bf16(p_i) < 16 bf16(p_j+1)] * e_i     (e folded via a bf16
            sentinel in the masked preds broadcast; DVE tensor_scalar, 4x)
  A_e[j,i] = [16 dmask_i < 16 d_j]  (dmask = e ? d : 1e6; only on diagonal
            chunks; ScalarE sigmoid(BIG*(d16_j - d16mask_i)), accum_out
            gives the diagonal num_pairs partial)
  J = A_e * We on diagonal chunks (DVE tensor_tensor, bf16 2x); J = We on
            below-chunks (free).
  PSUM += sum_j J * [p_hi_j, p_lo_j, 1]  via TensorE matmuls, col-tiled so a
            whole macro-chunk accumulates in one fresh psum bank
            (p_hi + p_lo = f32 preds split into two bf16 for precision).

  Host: loss_sum = sum_i S1e_i + (1 - p_i) S0e_i (in sorted space),
  num_pairs = sum(diagonal accums) + 128 * sum_t prefix_eones[below(t)]
  (exact integers).  The p-compare runs in bf16: any pair it can misclassify
  has |hinge| <= one bf16 ulp, so loss error stays ~1e-4 relative; the
  duration compare is exact except for saturated-sigmoid boundary pairs
  (|d_i-d_j| < ~1e-7 d) and rank ties exactly at chunk boundaries, both
  O(1e-6) relative.
"""

import numpy as np
import ml_dtypes

N = 8192
NCORES = 8
JB = N // NCORES          # j's per core = 1024
NT = JB // 128            # j-tiles per core = 8
CH = 2048                 # i macro-chunk width
NCH = N // CH             # 4
SUB = 512                 # matmul N / psum bank width (f32)
NSUB = CH // SUB          # 4
BCH = 1024                # broadcast psum chunk width
REP = 16                  # host-replicated rows for the broadcast matmul
BIG = np.float32(1.0e30)
DMASK_FILL = np.float32(1.0e6)   # finite sentinel > any duration
PSENT = np.float32(1.0e30)       # bf16 sentinel > any 16*(p+1)
BF16 = ml_dtypes.bfloat16

_CACHE = {}


def _tile_rank0(c, t):
    """First sorted rank covered by core c's j-tile t."""
    return 1024 * t + 128 * c


def _build_module():
    import concourse.bass as bass
    import concourse.bacc as bacc
    import concourse.tile as tile
    from concourse import mybir

    f32 = mybir.dt.float32
    bf16 = mybir.dt.bfloat16
    Alu = mybir.AluOpType
    Act = mybir.ActivationFunctionType

    nc = bacc.Bacc(trn_type="TRN2")
    t_dm = nc.dram_tensor("dmask16", [REP, N], f32, kind="ExternalInput")
    t_pe = nc.dram_tensor("pebf16", [REP, N], bf16, kind="ExternalInput")
    # djcols: [:, 0:NT] = 16*dj, [:, NT:2NT] = BIG*16*dj, [:, 2NT:3NT] = 16*bf16(p_j+1)
    t_djcols = nc.dram_tensor("djcols", [128, 3 * NT], f32, kind="ExternalInput")
    # pcols: lhst per t, zero-padded to 32 cols ([p_hi|p_lo|1|0...]) so the
    # start=True matmul initializes the full 32-partition psum group.
    t_pcols = nc.dram_tensor("pcols", [128, 32 * NT], bf16, kind="ExternalInput")
    t_outj = nc.dram_tensor("outj", [NCH, NSUB, 3, SUB], f32, kind="ExternalOutput")
    t_outra = nc.dram_tensor("outra", [128, NT], f32, kind="ExternalOutput")

    with tile.TileContext(nc) as tc:
        with (
            tc.tile_pool(name="consts", bufs=1) as consts,
            tc.tile_pool(name="bcast", bufs=1) as bcast,
            tc.tile_pool(name="awork", bufs=3) as awork,
            tc.tile_pool(name="wwork", bufs=4) as wwork,
            tc.tile_pool(name="jwork", bufs=3) as jwork,
            tc.tile_pool(name="stage", bufs=2) as stagep,
            tc.tile_pool(name="scratch", bufs=1) as scratch,
            tc.tile_pool(name="bps", bufs=2, space="PSUM") as bpsp,
            tc.tile_pool(name="acc", bufs=4, space="PSUM") as accp,
        ):
            djcols_s = consts.tile([128, 3 * NT], f32, tag="djcols")
            pcols_s = consts.tile([128, 32 * NT], bf16, tag="pcols")
            dmrows = consts.tile([REP, N], f32, tag="dmrows")
            perows = consts.tile([REP, N], bf16, tag="perows")
            ones_f = consts.tile([REP, 128], f32, tag="ones_f")
            ones_b = consts.tile([REP, 128], bf16, tag="ones_b")
            r_act = consts.tile([128, NT], f32, tag="ract")

            nc.sync.dma_start(djcols_s[:], t_djcols[:])
            nc.sync.dma_start(pcols_s[:], t_pcols[:])
            # Column-split the replicated-row loads: one DMA per macro-chunk
            # so they spread over queues and chunk C's broadcast matmuls wait
            # on exactly one DMA semaphore.
            for b in range(N // BCH):
                csl = slice(b * BCH, (b + 1) * BCH)
                nc.sync.dma_start(dmrows[:, csl], t_dm[:, csl])
                nc.sync.dma_start(perows[:, csl], t_pe[:, csl])
            nc.vector.memset(ones_f[:], 1.0)
            nc.vector.memset(ones_b[:], 1.0)

            # Tiny warm-up copies so the big ops don't accumulate DMA waits.
            warm_a = scratch.tile([128, 1], f32, tag="warm_a")
            warm_v = scratch.tile([128, 1], bf16, tag="warm_v")
            nc.scalar.copy(warm_a[:], djcols_s[:, 0:1])
            nc.vector.tensor_copy(warm_v[:], pcols_s[:, 0:1])

            # Broadcast i-axis vectors across partitions: K=REP PE matmul
            # (ones.T @ rows -> 16x-scaled values in PSUM), engine copy to
            # SBUF.  All d-chunks first (ScalarE copies), then p-chunks (DVE
            # copies), so every psum-slot reuse is one engine semaphore.
            pbc = [
                bcast.tile([128, CH], bf16, tag=f"pbc{C}", name=f"pbc{C}")
                for C in range(NCH)
            ]
            HB = CH // 2  # 1024: half-chunk; tile t's exact-compare region
            first = True
            for C in range(NCH):
                # p-broadcast for this chunk (later chunks overlap the
                # previous chunk's compute).
                for h in range(CH // BCH):
                    off = C * CH + h * BCH
                    bp2 = bpsp.tile([128, BCH], f32, tag="bps")
                    if first:
                        # Dummy 1x1 matmuls: advance PE's vector clock past
                        # the memsets and row DMAs one semaphore at a time.
                        for wlhs, wrhs in (
                            (ones_b, ones_b),
                            (ones_f, dmrows),
                            (ones_b, perows),
                        ):
                            nc.tensor.matmul(
                                bp2[0:1, 0:1], wlhs[0:1, 0:1], wrhs[0:1, 0:1],
                                start=True, stop=True,
                            )
                        first = False
                    for s in range(BCH // SUB):
                        nc.tensor.matmul(
                            bp2[:, s * SUB : (s + 1) * SUB],
                            ones_b[:],
                            perows[:, off + s * SUB : off + (s + 1) * SUB],
                            start=True,
                            stop=True,
                        )
                    if h % 2 == 0:
                        nc.vector.tensor_copy(pbc[C][:, h * BCH : (h + 1) * BCH], bp2[:])
                    else:
                        nc.scalar.copy(pbc[C][:, h * BCH : (h + 1) * BCH], bp2[:])

                ps_c = accp.tile([128, SUB], f32, tag="acc")
                # Emission order: fully-below tiles first (rhs = We only, so
                # PE streams them while the diagonal compare round-trips
                # through ScalarE/DVE), then the two diagonal tiles.
                order = list(range(2 * C + 2, NT)) + [2 * C, 2 * C + 1]
                first_01 = next(t for t in order if True)  # first writes subs 0,1
                first_23 = next(t for t in order if t != 2 * C)
                we_cache = {}
                for t in order:
                    diag = t // 2 == C
                    even = t % 2 == 0
                    wewidth = HB if (diag and even) else CH
                    we_t = wwork.tile([128, CH], bf16, tag="we")
                    nc.vector.tensor_scalar(
                        we_t[:, :wewidth],
                        pbc[C][:, :wewidth],
                        djcols_s[:, 2 * NT + t : 2 * NT + t + 1],
                        None,
                        Alu.is_lt,
                    )
                    if diag:
                        # In-loop d-broadcast into PSUM; ScalarE reads the
                        # exact duration compare input straight from PSUM.
                        hsl = slice(0, HB) if even else slice(HB, CH)
                        off = C * CH + (0 if even else HB)
                        bp_d = bpsp.tile([128, BCH], f32, tag="bps")
                        for s2 in range(BCH // SUB):
                            nc.tensor.matmul(
                                bp_d[:, s2 * SUB : (s2 + 1) * SUB],
                                ones_f[:],
                                dmrows[:, off + s2 * SUB : off + (s2 + 1) * SUB],
                                start=True,
                                stop=True,
                            )
                        a_t = awork.tile([128, HB], bf16, tag="a")
                        nc.scalar.activation(
                            a_t[:],
                            bp_d[:],
                            Act.Sigmoid,
                            bias=djcols_s[:, NT + t : NT + t + 1],
                            scale=-float(BIG),
                            accum_out=r_act[:, t : t + 1],
                        )
                        j_t = jwork.tile([128, HB], bf16, tag="j")
                        nc.vector.tensor_tensor(
                            j_t[:], a_t[:], we_t[:, hsl], Alu.mult
                        )
                        if even:
                            rhs_by_sub = [j_t[:, 0:SUB], j_t[:, SUB:HB], None, None]
                        else:
                            rhs_by_sub = [
                                we_t[:, 0:SUB],
                                we_t[:, SUB:HB],
                                j_t[:, 0:SUB],
                                j_t[:, SUB:HB],
                            ]
                    else:
                        rhs_by_sub = [
                            we_t[:, s * SUB : (s + 1) * SUB] for s in range(NSUB)
                        ]
                    for s in range(NSUB):
                        if rhs_by_sub[s] is None:
                            continue
                        nc.tensor.matmul(
                            ps_c[32 * s : 32 * s + 32, :],
                            pcols_s[:, 32 * t : 32 * t + 32],
                            rhs_by_sub[s],
                            start=(t == (first_01 if s < 2 else first_23)),
                            stop=(t == 2 * C + 1),
                            tile_position=(0, 32 * s),
                            # CoreSim's zero-region tracker mis-scales
                            # partition offsets of sliced psum tensors and
                            # reports false conflicts for M=32 col-tiled
                            # groups; each 32-partition group has exactly one
                            # start and one stop in PE order.
                            skip_group_check=True,
                        )
                st = stagep.tile([128, SUB], f32, tag="st")
                nc.scalar.copy(st[:], ps_c[:])
                for s in range(NSUB):
                    nc.sync.dma_start(t_outj[C, s], st[32 * s : 32 * s + 3, :])

            nc.sync.dma_start(t_outra[:], r_act[:])

    nc.finalize()  # Bacc: legalizes sync waits (event semaphores) + compiles
    return nc


def get_module():
    if "nc" not in _CACHE:
        _CACHE["nc"] = _build_module()
    return _CACHE["nc"]


def _sort_inputs(preds, targets):
    preds = np.asarray(preds, dtype=np.float32)
    targets = np.asarray(targets, dtype=np.float32)
    d = np.ascontiguousarray(targets[:, 0])
    e = np.ascontiguousarray(targets[:, 1])
    order = np.argsort(d, kind="stable")
    return preds[order], d[order], e[order]


def make_in_maps(preds, targets):
    p_s, d_s, e_s = _sort_inputs(preds, targets)

    dmask = np.where(e_s == 1.0, d_s, DMASK_FILL).astype(np.float32)
    pe_masked = np.where(e_s == 1.0, p_s.astype(BF16), PSENT.astype(BF16))
    dmask16 = np.ascontiguousarray(np.tile(dmask, (REP, 1)))
    pebf16 = np.ascontiguousarray(np.tile(pe_masked, (REP, 1)))

    in_maps = []
    for c in range(NCORES):
        dj = np.empty((128, NT), np.float32)
        pj = np.empty((128, NT), np.float32)
        for t in range(NT):
            r0 = _tile_rank0(c, t)
            dj[:, t] = d_s[r0 : r0 + 128]
            pj[:, t] = p_s[r0 : r0 + 128]
        dj16 = (np.float32(REP) * dj).astype(np.float32)   # exact (x16)
        djbig = (BIG * dj16).astype(np.float32)
        pj1_16 = ((pj + np.float32(1.0)).astype(BF16).astype(np.float32)
                  * np.float32(REP)).astype(np.float32)     # exact x16 of bf16, as f32
        djcols = np.concatenate([dj16, djbig, pj1_16], axis=1)
        phi = pj.astype(BF16)
        plo = (pj - phi.astype(np.float32)).astype(BF16)
        lhst = np.zeros((128, NT, 32), BF16)
        lhst[:, :, 0] = phi
        lhst[:, :, 1] = plo
        lhst[:, :, 2] = np.float32(1.0)
        pcols = lhst.reshape(128, 32 * NT)
        in_maps.append(
            {
                "dmask16": dmask16,
                "pebf16": pebf16,
                "djcols": np.ascontiguousarray(djcols),
                "pcols": np.ascontiguousarray(pcols),
            }
        )
    return in_maps


def combine_outputs(preds, targets, results):
    """results: per-core dicts with outj [NCH,NSUB,3,SUB], outra [128,NT]."""
    p_s, d_s, e_s = _sort_inputs(preds, targets)
    p64 = p_s.astype(np.float64)

    S1e = np.zeros(N, dtype=np.float64)
    S0e = np.zeros(N, dtype=np.float64)
    pairs = 0.0
    for res in results:
        outj = np.asarray(res["outj"], dtype=np.float64)
        S1e += (outj[:, :, 0, :] + outj[:, :, 1, :]).reshape(N)
        S0e += outj[:, :, 2, :].reshape(N)
        pairs += float(np.asarray(res["outra"], dtype=np.float64).sum())

    # Below-diagonal num_pairs term: each j of tile t sees all event-i's with
    # rank below its half-chunk boundary 1024*t (the device's exact compare
    # covers [1024 t, 1024 (t+1)) and above is all-zero).
    eones_prefix = np.concatenate([[0.0], np.cumsum(e_s == 1.0)])
    for t in range(NT):
        pairs += NCORES * 128 * float(eones_prefix[1024 * t])

    loss_sum = float(np.sum(S1e + (1.0 - p64) * S0e))
    if pairs > 0:
        out = loss_sum / max(pairs, 1.0)
    else:
        out = 0.0
    return np.float32(out)


def kernel(preds, targets):
    from concourse.bass_utils import run_bass_kernel_spmd

    nc = get_module()
    in_maps = make_in_maps(preds, targets)
    res = run_bass_kernel_spmd(nc, in_maps, core_ids=list(range(NCORES)))
    return combine_outputs(preds, targets, res.results)
